# revision 10
# baseline (speedup 1.0000x reference)
"""DigitCaps dynamic-routing kernel for 8 TRN2 NeuronCores.

Reference computation (f32):
    u_hat[b,r,j,d] = sum_c W[r,j,d,c] * x[b,r,c]
    b_ij = 0
    for it in 1..3:
        c = softmax_j(b_ij)                       # [R, J]
        s[b,j,d] = sum_r c[r,j] u_hat[b,r,j,d]
        v = squash(s)                             # [B, J, D]
        b_ij += mean_b sum_d u_hat[b,r,j,d] v[b,j,d]
    return v[..., None]

Strategy: shard the R=1152 routes across 8 cores (144 each), full batch on
every core.  u_hat is never materialized; both big contractions go through
the rank-8 factorization:
    s[b,jd]   = sum_{rc} x[b,(rc)] * (c[r,j] W[(rc),(jd)])     (PE matmuls)
    a[r,j]    = sum_{cd} W[(rc),(jd)] * Q[(rc),(jd)]           (DVE + PE)
      where Q[(rc),(jd)] = (1/B) sum_b x[b,(rc)] v[b,(jd)]     (PE matmuls)
Per-iteration collective: one 80KB AllReduce of the partial s.
Host pre-arranges all device layouts so every DMA is contiguous.
"""

import sys

sys.path.insert(0, "/opt/trn_rl_repo")

import numpy as np

B = 128          # batch
R = 1152         # num_routes
J = 10           # num_caps
D = 16           # caps_dim
C = 8            # caps_in
NUM_IT = 3
N_CORES = 8
RG = 9           # route groups per core (16 routes x 8 c = 128 partitions)
RPC = R // N_CORES  # 144 routes per core
JD = J * D       # 160

_CACHE = {}


def _build_bass():
    import concourse.bacc as bacc
    import concourse.mybir as mybir
    from concourse.tile import TileContext

    f32 = mybir.dt.float32
    nc = bacc.Bacc(None, target_bir_lowering=False, num_devices=N_CORES)

    xrc = nc.dram_tensor("xrc", [128, RG * B], f32, kind="ExternalInput")
    xt = nc.dram_tensor("xt", [B, RG * 128], f32, kind="ExternalInput")
    w = nc.dram_tensor("w", [128, RG * JD], f32, kind="ExternalInput")
    bpat = nc.dram_tensor("bpat", [128, 128], f32, kind="ExternalInput")
    out = nc.dram_tensor("out", [B, JD], f32, kind="ExternalOutput")

    # collective bounce buffers (DRAM)
    s_in = nc.dram_tensor("s_in", [B, JD], f32)
    s_out = nc.dram_tensor("s_out", [B, JD], f32, addr_space="Shared")

    groups = [list(range(N_CORES))]

    with TileContext(nc) as tc:
        with (
            tc.tile_pool(name="inp", bufs=1) as inp,
            tc.tile_pool(name="work", bufs=2) as work,
            tc.tile_pool(name="small", bufs=1) as small,
            tc.tile_pool(name="psum", bufs=2, space="PSUM") as psum,
            tc.tile_pool(name="qpsum", bufs=4, space="PSUM") as qpsum,
        ):
            xrc_sb = inp.tile([128, RG * B], f32, tag="xrc")
            xt_sb = inp.tile([B, RG * 128], f32, tag="xt")
            w_sb = inp.tile([128, RG * JD], f32, tag="w")
            bpat_sb = inp.tile([128, 128], f32, tag="bpat")
            nc.sync.dma_start(out=xrc_sb[:], in_=xrc[:])
            nc.sync.dma_start(out=xt_sb[:], in_=xt[:])
            nc.sync.dma_start(out=w_sb[:], in_=w[:])
            nc.sync.dma_start(out=bpat_sb[:], in_=bpat[:])

            cw_sb = small.tile([128, RG * JD], f32, tag="cw")
            b_sb = small.tile([128, RG * J], f32, tag="bij")
            c_sb = small.tile([128, RG * J], f32, tag="cij")
            t_sb = small.tile([128, RG * J], f32, tag="t")
            v_sb = small.tile([B, JD], f32, tag="v")
            s_sb = small.tile([B, JD], f32, tag="s")
            # softmax/squash temporaries
            m_sb = small.tile([128, RG], f32, tag="m")
            e_sb = small.tile([128, RG * J], f32, tag="e")
            sqn_sb = small.tile([B, J], f32, tag="sqn")
            rt_sb = small.tile([B, J], f32, tag="rt")
            den_sb = small.tile([B, J], f32, tag="den")
            fac_sb = small.tile([B, J], f32, tag="fac")
            s2_sb = small.tile([B, JD], f32, tag="s2")

            def softmax():
                """c_sb = softmax_j(b_sb), per (partition, g)."""
                bv = b_sb[:].rearrange("p (g j) -> p g j", g=RG, j=J)
                cv = c_sb[:].rearrange("p (g j) -> p g j", g=RG, j=J)
                ev = e_sb[:].rearrange("p (g j) -> p g j", g=RG, j=J)
                nc.vector.tensor_reduce(
                    out=m_sb[:], in_=bv, axis=mybir.AxisListType.X,
                    op=mybir.AluOpType.max,
                )
                mb = m_sb[:].unsqueeze(-1).broadcast_to([128, RG, J])
                nc.vector.tensor_tensor(
                    out=ev, in0=bv, in1=mb, op=mybir.AluOpType.subtract
                )
                nc.scalar.activation(
                    out=e_sb[:], in_=e_sb[:], func=mybir.ActivationFunctionType.Exp
                )
                nc.vector.tensor_reduce(
                    out=m_sb[:], in_=ev, axis=mybir.AxisListType.X,
                    op=mybir.AluOpType.add,
                )
                nc.vector.reciprocal(out=m_sb[:], in_=m_sb[:])
                rb = m_sb[:].unsqueeze(-1).broadcast_to([128, RG, J])
                nc.vector.tensor_tensor(
                    out=cv, in0=ev, in1=rb, op=mybir.AluOpType.mult
                )

            def compute_cw():
                """cw_sb = w_sb * c (broadcast over d)."""
                wv = w_sb[:].rearrange("p (g j d) -> p g j d", g=RG, j=J, d=D)
                cwv = cw_sb[:].rearrange("p (g j d) -> p g j d", g=RG, j=J, d=D)
                cb = (
                    c_sb[:]
                    .rearrange("p (g j) -> p g j", g=RG, j=J)
                    .unsqueeze(-1)
                    .broadcast_to([128, RG, J, D])
                )
                nc.vector.tensor_tensor(out=cwv, in0=wv, in1=cb,
                                        op=mybir.AluOpType.mult)

            def s_matmuls(rhs_sb):
                s_ps = psum.tile([128, JD], f32, tag="s_ps")
                xv = xrc_sb[:].rearrange("p (g b) -> p g b", g=RG, b=B)
                rv = rhs_sb[:].rearrange("p (g f) -> p g f", g=RG, f=JD)
                for g in range(RG):
                    nc.tensor.matmul(
                        s_ps[:], lhsT=xv[:, g, :], rhs=rv[:, g, :],
                        start=(g == 0), stop=(g == RG - 1),
                    )
                return s_ps

            def squash(scale):
                """v_sb = squash(scale * s_sb)."""
                sv = s_sb[:].rearrange("b (j d) -> b j d", j=J, d=D)
                s2v = s2_sb[:].rearrange("b (j d) -> b j d", j=J, d=D)
                vv = v_sb[:].rearrange("b (j d) -> b j d", j=J, d=D)
                # s2 = (scale*s)^2 ; sqn = sum_d s2
                nc.scalar.activation(
                    out=s2_sb[:], in_=s_sb[:],
                    func=mybir.ActivationFunctionType.Square, scale=float(scale),
                )
                nc.vector.tensor_reduce(
                    out=sqn_sb[:], in_=s2v, axis=mybir.AxisListType.X,
                    op=mybir.AluOpType.add,
                )
                # factor = sqrt(sqn) / (1 + sqn)
                nc.scalar.activation(
                    out=rt_sb[:], in_=sqn_sb[:],
                    func=mybir.ActivationFunctionType.Sqrt,
                )
                nc.vector.tensor_scalar_add(out=den_sb[:], in0=sqn_sb[:],
                                            scalar1=1.0)
                nc.vector.reciprocal(out=den_sb[:], in_=den_sb[:])
                nc.vector.tensor_tensor(out=fac_sb[:], in0=rt_sb[:],
                                        in1=den_sb[:], op=mybir.AluOpType.mult)
                # v = (scale*s) * factor  -> fold scale into factor via s2? no:
                # fold by scaling factor once more: v = s * (scale*factor)
                nc.vector.tensor_scalar_mul(out=fac_sb[:], in0=fac_sb[:],
                                            scalar1=float(scale))
                fb = fac_sb[:].unsqueeze(-1).broadcast_to([B, J, D])
                nc.vector.tensor_tensor(out=vv, in0=sv, in1=fb,
                                        op=mybir.AluOpType.mult)

            def agreement(first):
                """t_sb[p,(g,j)] = sum_d W*Q;  b_sb += bpat^T @ t (c-sum,
                broadcast over c partitions, * 1/B folded into bpat)."""
                wv = w_sb[:].rearrange("p (g f) -> p g f", g=RG, f=JD)
                tv = t_sb[:].rearrange("p (g j) -> p g j", g=RG, j=J)
                for g in range(RG):
                    q_ps = qpsum.tile([128, JD], f32, tag="q_ps")
                    nc.tensor.matmul(
                        q_ps[:], lhsT=xt_sb[:, g * 128:(g + 1) * 128],
                        rhs=v_sb[:], start=True, stop=True,
                    )
                    prod = work.tile([128, JD], f32, tag="prod")
                    pv = prod[:].rearrange("p (j d) -> p j d", j=J, d=D)
                    nc.vector.tensor_tensor(
                        out=prod[:], in0=wv[:, g, :],
                        in1=q_ps[:], op=mybir.AluOpType.mult,
                    )
                    nc.vector.tensor_reduce(
                        out=tv[:, g, :], in_=pv, axis=mybir.AxisListType.X,
                        op=mybir.AluOpType.add,
                    )
                a_ps = psum.tile([128, RG * J], f32, tag="a_ps")
                nc.tensor.matmul(a_ps[:], lhsT=bpat_sb[:], rhs=t_sb[:],
                                 start=True, stop=True)
                if first:
                    nc.vector.tensor_copy(out=b_sb[:], in_=a_ps[:])
                else:
                    nc.vector.tensor_tensor(out=b_sb[:], in0=b_sb[:],
                                            in1=a_ps[:],
                                            op=mybir.AluOpType.add)

            for it in range(NUM_IT):
                if it == 0:
                    s_ps = s_matmuls(w_sb)  # c uniform: fold 1/J into squash
                    scale = 1.0 / J
                else:
                    softmax()
                    compute_cw()
                    s_ps = s_matmuls(cw_sb)
                    scale = 1.0
                # partial s -> AllReduce -> full s
                nc.scalar.copy(out=s_sb[:], in_=s_ps[:])
                nc.sync.dma_start(out=s_in[:], in_=s_sb[:])
                nc.gpsimd.collective_compute(
                    "AllReduce", mybir.AluOpType.add, replica_groups=groups,
                    ins=[s_in[:]], outs=[s_out[:]],
                )
                nc.sync.dma_start(out=s_sb[:], in_=s_out[:])
                squash(scale)
                if it < NUM_IT - 1:
                    agreement(first=(it == 0))

            nc.sync.dma_start(out=out[:], in_=v_sb[:])

    nc.finalize()
    return nc


def _prep_inputs(x, W):
    """Build per-core contiguous SBUF images."""
    x = np.ascontiguousarray(x, dtype=np.float32)
    W0 = np.ascontiguousarray(W.reshape(R, J, D, C), dtype=np.float32)
    # W0t[r, c, j, d]
    W0t = W0.transpose(0, 3, 1, 2)
    # (k, g, r16, c, j, d) -> (k, (r16, c), (g, j, d))
    w_img = np.ascontiguousarray(
        W0t.reshape(N_CORES, RG, 16, C, J, D)
        .transpose(0, 2, 3, 1, 4, 5)
        .reshape(N_CORES, 128, RG * JD)
    )
    xr = x.reshape(B, N_CORES, RG, 16, C)
    # (k, r16, c, g, b)
    xrc_img = np.ascontiguousarray(
        xr.transpose(1, 3, 4, 2, 0).reshape(N_CORES, 128, RG * B)
    )
    # (k, b, g, r16, c)
    xt_img = np.ascontiguousarray(
        xr.transpose(1, 0, 2, 3, 4).reshape(N_CORES, B, RG * 128)
    )
    p = np.arange(128)
    bpat = np.where((p[:, None] // C) == (p[None, :] // C), 1.0 / B, 0.0).astype(
        np.float32
    )
    return w_img, xrc_img, xt_img, bpat


TRACE = False  # set True (e.g. from test.py) to capture HW profile/exec time
TRACE_DIR = None


def last_exec_time_ns():
    return _CACHE.get("exec_time_ns")


def kernel(input, W):
    from concourse.bass_utils import run_bass_kernel_spmd

    if "nc" not in _CACHE:
        _CACHE["nc"] = _build_bass()
    nc = _CACHE["nc"]

    w_img, xrc_img, xt_img, bpat = _prep_inputs(
        np.asarray(input), np.asarray(W)
    )
    in_maps = [
        {
            "xrc": xrc_img[k],
            "xt": xt_img[k],
            "w": w_img[k],
            "bpat": bpat,
        }
        for k in range(N_CORES)
    ]
    res = run_bass_kernel_spmd(
        nc, in_maps, list(range(N_CORES)), trace=TRACE, tmpdir=TRACE_DIR
    )
    _CACHE["exec_time_ns"] = res.exec_time_ns
    _CACHE["profile_json"] = res.profile_json
    v = res.results[0]["out"]  # [B, J*D], identical on all cores
    return np.ascontiguousarray(v.reshape(B, J, D, 1).astype(np.float32))
